# revision 87
# baseline (speedup 1.0000x reference)
"""Trainium2 Bass kernel: per-species expert linear + structure segment-sum.

Math: out[g] = sum_{atoms i in structure g} (x[i] @ W[species_i] + b[species_i])
Since everything is linear, aggregate first, matmul after:
  A[g, s, :] = sum_{i: struct_i=g, species_i=s} [x[i] | 1]        (257-dim)
  out[g]     = sum_s A[g, s, :] @ [[W_s], [b_s]]                  (257 x 256)

Stage 1 (on device): segment-sum of [x | 1] rows by combined seg = 4*struct +
species, via one-hot matmuls on the TensorEngine accumulating in PSUM.
Atoms are pre-sorted by struct, so a 128-atom tile touches only ~8-20 segs;
masks are built at 64-seg-block granularity (DVE tensor_scalar of [128, W]
with W = 64 or 128) and each touched 64-block gets its own M=64 matmul into
the 128-seg PSUM window (out partition base 0/64). This cuts the DVE mask
cost vs full 128-wide masks -- DVE is the bottleneck engine. The
tile->block schedule comes from the actual indices, unioned across all 8
cores so the SPMD graph is identical on every core.

Stage 2 (on device): transpose window accumulators (PE transpose) and
contract the 257-dim feature axis against the packed expert weights,
emitted per pair of windows as soon as they are flushed. x streams in bf16
(one-hot is exact in bf16; rel err ~3e-3 total), PSUM accumulates f32.

Sharding: 25000 contiguous atoms per core (structs stay contiguous per core
because structural_indices are sorted); host overlap-adds the 8 partial
per-struct outputs. x is packed chunk-contiguous on host (each DMA src is
one linear DRAM block) and the x stream is issued alternately from the Sync
and Scalar HWDGE queues so descriptor generation never serializes.
"""

import numpy as np

P = 128
N_ATOMS = 200_000
D_IN = 256
D_OUT = 256
N_SPECIES = 4
N_STRUCT = 2_000
N_CORES = 8
SH = N_ATOMS // N_CORES            # atoms per core
TPC = (SH + P - 1) // P            # tiles per core
SH_PAD = TPC * P                   # padded atoms per core
CH = 7                             # max x tiles per DMA chunk
CHUNK_BUFS = 8
M_BUFS = 12
AT_BUFS = 2
TP_BUFS = 2
PO_BUFS = 2
DF = D_IN                          # features per tile (bias handled on host)
SENTINEL = 3.0e8                   # seg value for padded atoms (matches nothing)
B = 64                             # seg block granularity (psum bases 0/64 only)
MASK_FP8 = False                   # fp8e4 masks work (exact) but bench slower


def _chunk_plan():
    """Chunk start tiles: small head (fast pipeline fill), CH-sized body,
    small tail (short serial tail)."""
    starts = [0, 2, 4]
    t = 7
    while t + CH <= TPC - 7:
        starts.append(t)
        t += CH
    while t < TPC:
        starts.append(t)
        t += 2
    sizes = {s: (starts + [TPC])[i + 1] - s for i, s in enumerate(starts)}
    return starts, sizes


def _schedule(seg_local_real):
    """seg_local_real: list of per-core int arrays [SH] of local seg ids.
    Block-granular schedule, unioned across cores (identical SPMD graph).
    Returns dict with per-tile block ranges, per-block first/last tiles,
    window flush schedule, and PSUM pool sizing."""
    max_seg = max(int(s.max()) for s in seg_local_real)
    n_blocks = max_seg // B + 1
    NW = ((n_blocks + 1) // 2 + 3) // 4 * 4  # windows of 2 blocks, mult of 4

    b0 = np.full(TPC, 1 << 30, np.int64)
    b1 = np.full(TPC, -1, np.int64)
    for s in seg_local_real:
        for t in range(TPC):
            a0, a1 = t * P, min((t + 1) * P, SH)
            if a0 >= SH:
                break
            tl = s[a0:a1]
            b0[t] = min(b0[t], int(tl.min()) // B)
            b1[t] = max(b1[t], int(tl.max()) // B)
    assert int((b1 - b0).max()) < 2, "tile spans >2 seg blocks"

    first_b = {}
    last_b = {}
    for t in range(TPC):
        for b in range(int(b0[t]), int(b1[t]) + 1):
            if b not in first_b:
                first_b[b] = t
            last_b[b] = t

    win_first = {}
    win_last = {}
    for b in first_b:
        w = b // 2
        win_first[w] = min(win_first.get(w, 1 << 30), first_b[b])
        win_last[w] = max(win_last.get(w, -1), last_b[b])
    # untouched blocks inside touched windows -> zeroed at flush
    zero_blocks = {
        w: [b for b in range(2 * w, 2 * w + 2) if b not in first_b]
        for w in win_first
    }
    alive = max(
        sum(1 for w in win_first if win_first[w] <= t <= win_last[w])
        for t in range(TPC)
    )
    win_bufs = min(max(2, alive + 1), 4)
    return {
        "NW": NW,
        "b0": [int(v) for v in b0],
        "b1": [int(v) for v in b1],
        "first_b": first_b,
        "last_b": last_b,
        "win_first": win_first,
        "win_last": win_last,
        "zero_blocks": zero_blocks,
        "win_bufs": win_bufs,
    }


def _build(sched, reps=1):
    import contextlib

    import concourse.bacc as bacc
    import concourse.mybir as mybir
    import concourse.tile as tile

    f32 = mybir.dt.float32
    bf16 = mybir.dt.bfloat16
    mdt = mybir.dt.float8e4 if MASK_FP8 else bf16
    NW = sched["NW"]
    starts, sizes = _chunk_plan()
    nchunks = len(starts)

    nc = bacc.Bacc(None, target_bir_lowering=False)
    xp_d = nc.declare_dram_parameter("xp", [nchunks * P, CH * DF], bf16, isOutput=False)
    segs_d = nc.declare_dram_parameter("segs", [P, P + TPC], f32, isOutput=False)
    wk_d = nc.declare_dram_parameter("wk", [P, 8 * D_OUT], bf16, isOutput=False)
    id_d = nc.declare_dram_parameter("ident", [P, P], bf16, isOutput=False)
    out_d = nc.declare_dram_parameter("out", [NW * 32, D_OUT], f32, isOutput=True)

    with tile.TileContext(nc) as tc:
        with (
            tc.tile_pool(name="const", bufs=1) as constp,
            tc.tile_pool(name="chunk", bufs=CHUNK_BUFS) as chunkp,
            tc.tile_pool(name="onehot", bufs=M_BUFS) as mp,
            tc.tile_pool(name="atmp", bufs=AT_BUFS) as atp,
            tc.tile_pool(name="tks", bufs=1) as tkp,
            tc.tile_pool(name="win", bufs=sched["win_bufs"], space="PSUM") as winp,
            tc.tile_pool(name="tp", bufs=TP_BUFS, space="PSUM") as tpp,
            tc.tile_pool(name="po", bufs=PO_BUFS, space="PSUM") as pop,
        ):
            # f32 iota columns 0:P, segs columns P:P+TPC; split into a small
            # fast-completing first DMA (iota + first 32 tile columns, Sync)
            # and the rest on Scalar, so the mask pipeline starts ~1us sooner
            segio_sb = constp.tile([P, P + TPC], f32)
            nc.sync.dma_start(segio_sb[:, : P + 32], segs_d[:, : P + 32])
            nc.scalar.dma_start(segio_sb[:, P + 32 :], segs_d[:, P + 32 :])
            segs_sb = segio_sb
            iota_bf = constp.tile([P, P], bf16)
            nc.vector.tensor_copy(iota_bf[:], segio_sb[:, :P])
            iota_sb = iota_bf[:]
            ident_sb = constp.tile([P, P], bf16)
            wk_sb = constp.tile([P, 8 * D_OUT], bf16)
            zmask_sb = constp.tile([P, B], mdt)
            scratch_sb = constp.tile([1, 1], f32)

            tk0 = tkp.tile([P, NW * P], bf16, tag="tk0")
            tk1 = tkp.tile([P, NW * P], bf16, tag="tk1")

            loop_cm = (
                tc.For_i(
                    0,
                    reps,
                    1,
                    hint_engines=(
                        mybir.EngineType.PE,
                        mybir.EngineType.DVE,
                        mybir.EngineType.Activation,
                        mybir.EngineType.SP,
                    ),
                )
                if reps > 1
                else contextlib.nullcontext()
            )
            first_body = [True]
            with loop_cm:
                _emit_body(
                    nc, tc, mybir, f32, bf16, mdt, sched, starts, sizes,
                    chunkp, mp, atp, winp, tpp, pop,
                    segs_sb, iota_sb, ident_sb, wk_sb, zmask_sb,
                    scratch_sb, tk0, tk1, xp_d, out_d, id_d, wk_d,
                    first_body,
                )

    nc.compile()
    return nc


def _emit_body(
    nc, tc, mybir, f32, bf16, mdt, sched, starts, sizes,
    chunkp, mp, atp, winp, tpp, pop,
    segs_sb, iota_sb, ident_sb, wk_sb, zmask_sb,
    scratch_sb, tk0, tk1, xp_d, out_d, id_d, wk_d, first_body,
):
    NW = sched["NW"]
    NWG = NW // 4
    b0 = sched["b0"]
    b1 = sched["b1"]
    first_b = sched["first_b"]
    last_b = sched["last_b"]
    win_last = sched["win_last"]
    zero_blocks = sched["zero_blocks"]

    po_tiles = {}
    po_done = {g: 0 for g in range(NWG)}
    pairs_done = set()

    def emit_stage2_pair(w_lo):
        # windows (w_lo, w_lo+1) fill output partitions [64r, 64r+64) of
        # group g's psum, r = (w_lo//2) % 2 (PE out base must be 0/32/64)
        g, r = w_lo // 4, (w_lo // 2) % 2
        pairs_done.add(w_lo)
        if g not in po_tiles:
            po_tiles[g] = pop.tile([P, D_OUT], f32, tag="po", name=f"po{g}")
        po = po_tiles[g]
        blk = po[64 * r : 64 * r + 64, :]
        for kc, tkbuf in ((0, tk0), (1, tk1)):
            for s in range(N_SPECIES):
                nc.tensor.matmul(
                    blk,
                    lhsT=tkbuf[:, w_lo * P + s : (w_lo + 2) * P : 4],
                    rhs=wk_sb[:, (s * 2 + kc) * D_OUT : (s * 2 + kc + 1) * D_OUT],
                    start=(kc == 0 and s == 0),
                    stop=(kc == 1 and s == N_SPECIES - 1),
                )
        po_done[g] += 1
        if po_done[g] == 2:
            ob = atp.tile([P, D_OUT], f32, tag="ob", name=f"ob{g}")
            nc.scalar.copy(ob[:], po[:])
            nc.sync.dma_start(out_d[g * P : (g + 1) * P, :], ob[:])
            del po_tiles[g]

    psw = {}
    chunk = None
    coff = 0
    ci = -1
    for t in range(TPC):
        if t in sizes:
            ci += 1
            csz = sizes[t]
            chunk = chunkp.tile([P, CH * DF], bf16, tag="chunk", name=f"ch{t}")
            eng = nc.sync if ci % 2 == 0 else nc.scalar
            eng.dma_start(
                chunk[:, : csz * DF], xp_d[ci * P : (ci + 1) * P, : csz * DF]
            )
            coff = t
            if first_body[0]:
                if ci == 1:
                    # gpsimd is otherwise idle: zero-fill tk + zmask there
                    nc.gpsimd.memset(tk0[:], 0.0)
                    nc.gpsimd.memset(tk1[:], 0.0)
                    nc.gpsimd.memset(zmask_sb[:], 0.0)
                elif ci == 2:
                    nc.sync.dma_start(ident_sb[:], id_d[:])
                elif ci == 3:
                    # trigger the scalar-engine act table load early (1.3us)
                    # so it doesn't stall the first window flush
                    nc.scalar.copy(scratch_sb[:], segs_sb[:1, :1])
                elif ci == 4:
                    nc.sync.dma_start(wk_sb[:], wk_d[:])
                    first_body[0] = False
        xt = chunk[:, (t - coff) * DF : (t - coff + 1) * DF]
        nblk = b1[t] - b0[t] + 1
        m = mp.tile([P, P], mdt, tag="m")
        # m[a, j] = (iota[j] - seg[a] == -64*b0)  <=>  seg[a] == 64*b0 + j
        nc.vector.tensor_scalar(
            out=m[:, : nblk * B],
            in0=iota_sb[:, : nblk * B],
            scalar1=segs_sb[:, P + t : P + t + 1],
            scalar2=float(-(B * b0[t])),
            op0=mybir.AluOpType.subtract,
            op1=mybir.AluOpType.is_equal,
        )
        for b in range(b0[t], b1[t] + 1):
            w = b // 2
            if w not in psw:
                psw[w] = winp.tile([P, DF], f32, tag="win", name=f"win{w}")
            base = B * (b % 2)
            nc.tensor.matmul(
                psw[w][base : base + B, :],
                lhsT=m[:, (b - b0[t]) * B : (b - b0[t] + 1) * B],
                rhs=xt,
                start=(t == first_b[b]),
                stop=(t == last_b[b]),
            )
        # flush finished windows: transpose into feature-major buffers
        for w in sorted(psw):
            if t != win_last[w]:
                continue
            for b in zero_blocks[w]:
                base = B * (b % 2)
                nc.tensor.matmul(
                    psw[w][base : base + B, :],
                    lhsT=zmask_sb[:],
                    rhs=xt,
                    start=True,
                    stop=True,
                )
            at = atp.tile([P, DF], bf16, tag="at")
            nc.scalar.copy(at[:], psw[w][:])
            for kc, tkbuf in ((0, tk0), (1, tk1)):
                tp = tpp.tile([P, P], bf16, tag="tp")
                nc.tensor.transpose(
                    out=tp[:],
                    in_=at[:, kc * P : (kc + 1) * P],
                    identity=ident_sb[:],
                )
                nc.scalar.copy(tkbuf[:, w * P : (w + 1) * P], tp[:])
            del psw[w]
            # stage 2 for a window pair once its later window is flushed
            if w % 2 == 1:
                emit_stage2_pair(w - 1)

    # remaining pairs (NW padding / odd tail): zeros via memset tk columns
    for w_lo in range(0, NW, 2):
        if w_lo not in pairs_done:
            emit_stage2_pair(w_lo)


def _prep(x, W, b, central_species, structural_indices):
    """Host-side prep: schedule from indices + packed per-core in_maps."""
    import ml_dtypes

    bf16 = ml_dtypes.bfloat16
    x = np.asarray(x, dtype=np.float32)
    Wf = np.asarray(W, dtype=np.float32)
    bf = np.asarray(b, dtype=np.float32)
    cs = np.asarray(central_species).astype(np.int64)
    si = np.asarray(structural_indices).astype(np.int64)

    if not np.all(np.diff(si) >= 0):
        order = np.argsort(si, kind="stable")
        si = si[order]
        cs = cs[order]
        x = x[order]

    seg = 4 * si + cs
    # host-side bias term: sum over atoms of b[species], per structure
    counts = np.bincount(seg, minlength=4 * N_STRUCT).reshape(N_STRUCT, 4)
    bias_full = counts.astype(np.float32) @ bf
    g0 = [int(si[c * SH]) for c in range(N_CORES)]
    seg_local_real = [
        (seg[c * SH : (c + 1) * SH] - 4 * g0[c]).astype(np.int64)
        for c in range(N_CORES)
    ]
    sched = _schedule(seg_local_real)
    starts, sizes = _chunk_plan()

    iota = np.tile(np.arange(P, dtype=np.float32), (P, 1))
    ident = np.eye(P, dtype=bf16)
    wk = np.zeros((P, 8, D_OUT), bf16)
    for s in range(N_SPECIES):
        for kc in range(2):
            wk[:, s * 2 + kc, :] = Wf[s, kc * P : (kc + 1) * P, :].astype(bf16)
    wk = np.ascontiguousarray(wk.reshape(P, 8 * D_OUT))

    in_maps = []
    for c in range(N_CORES):
        xp = np.zeros((SH_PAD, DF), bf16)
        xp[:SH] = x[c * SH : (c + 1) * SH].astype(bf16)
        # partition-major within each chunk; chunks are contiguous DRAM
        # blocks so every DMA src is one linear region
        xp = xp.reshape(TPC, P, DF)
        xpk = np.zeros((len(starts) * P, CH * DF), bf16)
        for ci, t0 in enumerate(starts):
            csz = sizes[t0]
            blk = xp[t0 : t0 + csz].transpose(1, 0, 2).reshape(P, csz * DF)
            xpk[ci * P : (ci + 1) * P, : csz * DF] = blk
        segsT = np.full((TPC, P), SENTINEL, np.float32)
        segsT.reshape(-1)[:SH] = seg_local_real[c].astype(np.float32)
        segsT = np.ascontiguousarray(np.concatenate([iota, segsT.T], axis=1))
        in_maps.append(
            {"xp": xpk, "segs": segsT, "wk": wk, "ident": ident}
        )
    return {
        "build_args": (sched,),
        "in_maps": in_maps,
        "g0": g0,
        "NW": sched["NW"],
        "bias_full": bias_full,
    }


def kernel(x, W, b, central_species, structural_indices):
    from concourse.bass_utils import run_bass_kernel_spmd

    prep = _prep(x, W, b, central_species, structural_indices)
    nc = _build(*prep["build_args"])
    res = run_bass_kernel_spmd(
        nc, prep["in_maps"], core_ids=list(range(N_CORES))
    )

    g0, NW = prep["g0"], prep["NW"]
    full = np.zeros((N_STRUCT + NW * 32, D_OUT), np.float32)
    for c in range(N_CORES):
        full[g0[c] : g0[c] + NW * 32] += res.results[c]["out"]
    out = full[:N_STRUCT] + prep["bias_full"]
    return np.ascontiguousarray(out)
